# revision 24
# baseline (speedup 1.0000x reference)
"""Causal self-attention on 8 TRN2 NeuronCores.

Sharding: 8 cores = 4 batches x 2 head-groups (data parallel on B,
tensor parallel on heads). Core (b, g) computes batch b, heads
8g..8g+7 end-to-end (qkv slice -> causal attention -> partial
projection); the host sums the two per-batch partials.

v2 design (vs the f32r baseline):
- fp16 everywhere on-device (PSUM accumulation stays fp32). Halves all
  DMA and SBUF traffic at identical PE rate (1 cycle/row).
- Zero DRAM roundtrips: Q^T/K^T (qkT), V (vres), O^T (otj) and w_proj
  (wpt) all live in SBUF across phases; attention + projection read
  them directly.
- Softmax denominator without PE: DVE accumulates exp chunks into a
  [128,512] fp16 acc, one Pool partition_all_reduce broadcasts the
  cross-partition sum, DVE reciprocal + column-scale normalizes.
  Removes the 320 per-chunk "ones" matmuls (10.6% of PE work), the
  Ln/Exp ACT-table thrash (65 x 1.28us), and the rinv DMA broadcast.
- Causal mask is multiplicative-after-exp in fp16 (DVE 4x mode) rather
  than additive-in-PSUM (DVE 1x).
- Projection (phase D) is interleaved per 512-wide q-block into the
  next block's attention stream, so the PE never sits idle at a phase
  boundary; only the final q-block's projection is a (fully pipelined)
  tail.

Self-contained: hardcodes B=4, T=2048, C=2048, H=16, HD=128.
"""

import numpy as np

import concourse.bass as bass
import concourse.mybir as mybir
import concourse.tile as tile
from concourse import bacc
from concourse import bass_isa
from concourse.bass_utils import run_bass_kernel_spmd

B, T, C, H = 4, 2048, 2048, 16
HD = 128          # head dim
G = 2             # head groups (tensor parallel)
HPG = H // G      # 8 heads per core
DG = HPG * HD     # 1024 = per-core concat head dim
N_CORES = 8
SCALE = float(HD) ** -0.5

F32 = mybir.dt.float32
F16 = mybir.dt.float16

P = 128           # partitions
FN = 512          # moving free-dim per matmul (one PSUM bank of fp32)
CI = C // P       # 16 contraction chunks over C
TM = T // P       # 16 t chunks of 128
TN = T // FN      # 4 t chunks of 512
MQK = 2 * DG // P # 16 qk^T row chunks
NV = DG // FN     # 2 v column chunks of 512
NC_ = C // FN     # 4 proj col chunks of 512


def build_nc() -> bass.Bass:
    nc = bacc.Bacc()
    # Host pre-layouts (all fp16):
    #   xt   = x[b].T                        [C, T]
    #   wqk  [m, p, ci*128+col] = w_qk[ci*128+p, m*128+col]
    #   wv   [nv, p, ci*512+vc] = w_v[ci*128+p, nv*512+vc]
    #   wp   [p, hh, n]         = w_proj[g*DG + hh*128 + p, n]
    #   masks[r, mm, c] = 1.0 iff (c - r) >= 128*mm else 0.0
    xt = nc.declare_dram_parameter("xt", [C, T], F16, isOutput=False)
    wqk = nc.declare_dram_parameter("wqk", [MQK, P, CI * P], F16, isOutput=False)
    wv = nc.declare_dram_parameter("wv", [NV, P, CI * FN], F16, isOutput=False)
    wp = nc.declare_dram_parameter("wp", [P, HPG, C], F16, isOutput=False)
    masks = nc.declare_dram_parameter("masks", [P, P], F16, isOutput=False)
    out = nc.declare_dram_parameter("out", [T, C], F32, isOutput=True)

    Exp = mybir.ActivationFunctionType.Exp

    with tile.TileContext(nc) as tc:
        with (
            # resident across all phases: 64 + 32 + 16 + small = ~113 KB/partition
            tc.tile_pool(name="big", bufs=1) as big,
        ):
            qkT = big.tile([P, MQK, T], F16)        # Q^T rows m<8, K^T rows m>=8
            vres = big.tile([P, TM, HPG, HD], F16)  # V natural, [k, tm, h, d]
            masks_sb = big.tile([P, P], F16)        # triangle: 1 iff c >= r
            wpt_lo = big.tile([P, HPG, C // 2], F16)  # proj cols 0..1023
            ones16 = big.tile([P, P], F16)          # partition-sum stationary
            warm = big.tile([P, 1], F32)
            nc.gpsimd.memset(ones16[:], 1.0)
            nc.gpsimd.dma_start(masks_sb[:], masks[:, :])
            # pre-warm the Exp ACT table during the load phase so phase C
            # doesn't pay the 1.3us table load at the B->C seam
            nc.scalar.activation(warm[:], ones16[:, 0:1], Exp)
            # loaded during phase B on the gpsimd SWDGE queue
            for hh in range(HPG):
                nc.gpsimd.dma_start(wpt_lo[:, hh, :], wp[:, hh, 0:C // 2])

            # ---------- Phase B: V = x @ wv, then qk^T = wqk.T @ x.T ----------
            with (
                tc.tile_pool(name="xTp", bufs=1) as xTp,
                tc.tile_pool(name="wvp", bufs=4) as wvp,
                tc.tile_pool(name="wqp", bufs=2) as wqp,
                tc.tile_pool(name="bps", bufs=4, space="PSUM") as bps,
            ):
                xT = xTp.tile([P, CI, T], F16)  # x^T resident: 64KB/partition
                for ci in range(CI):
                    nc.sync.dma_start(xT[:, ci, :], xt[ci * P:(ci + 1) * P, :])

                # V pass: out[t, v] accumulated over ci
                for nv in range(NV):
                    wvt = {}
                    for q in range(4):  # quarter-ci tiles, ACT HWDGE queue
                        t_ = wvp.tile([P, 4, FN], F16, tag="wvt", name=f"wvt{nv}_{q}")
                        nc.scalar.dma_start(
                            t_[:],
                            wv[nv, :, q * 4 * FN:(q + 1) * 4 * FN]
                            .rearrange("p (ci n) -> p ci n", ci=4),
                        )
                        wvt[q] = t_
                    for tm in range(TM):
                        ps = bps.tile([P, FN], F32, tag="v", name=f"vps{nv}_{tm}")
                        for ci in range(CI):
                            nc.tensor.matmul(
                                ps[:], xT[:, ci, tm * P:(tm + 1) * P],
                                wvt[ci // 4][:, ci % 4, :],
                                start=(ci == 0), stop=(ci == CI - 1),
                            )
                        nc.scalar.copy(vres[:, tm, 4 * nv:4 * (nv + 1), :], ps[:])

                # qk pass: out[qkrow, t] accumulated over ci
                for m in range(MQK):
                    wq = wqp.tile([P, CI, P], F16, tag="wq", name=f"wq{m}")
                    nc.sync.dma_start(
                        wq[:], wqk[m, :, :].rearrange("p (ci n) -> p ci n", ci=CI)
                    )
                    for nt in range(TN):
                        ps = bps.tile([P, FN], F32, tag="qk")
                        for ci in range(CI):
                            nc.tensor.matmul(
                                ps[:], wq[:, ci, :], xT[:, ci, nt * FN:(nt + 1) * FN],
                                start=(ci == 0), stop=(ci == CI - 1),
                            )
                        nc.scalar.copy(qkT[:, m, nt * FN:(nt + 1) * FN], ps[:])

            # ---------- Phase C (attention) + interleaved Phase D (proj) ----------
            with (
                tc.tile_pool(name="whp", bufs=1) as whp,
                tc.tile_pool(name="pttp", bufs=2) as pttp,
                tc.tile_pool(name="accp", bufs=2) as accp,
                tc.tile_pool(name="redp", bufs=2) as redp,
                tc.tile_pool(name="otjp", bufs=2) as otjp,
                tc.tile_pool(name="dstg", bufs=4) as dstg,
                tc.tile_pool(name="cps", bufs=2, space="PSUM") as cps,
                tc.tile_pool(name="ops", bufs=2, space="PSUM") as ops_pool,
                tc.tile_pool(name="dps", bufs=2, space="PSUM") as dps,
            ):
                wpt_hi = whp.tile([P, HPG, C // 2], F16)  # proj cols 1024..2047
                for hh in range(HPG):
                    nc.gpsimd.dma_start(wpt_hi[:, hh, :], wp[:, hh, C // 2:])

                otj_tiles = {}

                def emit_dgroup(jp, g):
                    # one [128 t, 512 c] proj output tile, accumulated over heads
                    n, tmq = divmod(g, 4)
                    ps = dps.tile([P, FN], F32, tag="dg", name=f"dg{jp}_{g}")
                    wsrc = wpt_lo if n < 2 else wpt_hi
                    nn = n % 2
                    otj = otj_tiles[jp]
                    for h in range(HPG):
                        nc.tensor.matmul(
                            ps[:],
                            otj[:, h, tmq * P:(tmq + 1) * P],
                            wsrc[:, h, nn * FN:(nn + 1) * FN],
                            start=(h == 0), stop=(h == HPG - 1),
                        )
                    st = dstg.tile([P, FN], F32, tag="st", name=f"st{jp}_{g}")
                    nc.scalar.copy(st[:], ps[:])
                    tg = 4 * jp + tmq
                    nc.sync.dma_start(
                        out[tg * P:(tg + 1) * P, n * FN:(n + 1) * FN], st[:]
                    )

                with nc.allow_low_precision(
                    "fp16 softmax: exp in [0, 245], row sums < 4e3, "
                    "fp16 mantissa gives ~1e-4 relative accuracy"
                ):
                    LOOKP = 1  # lookahead in chunk PAIRS (2-bank PSUM tiles)
                    d_pending = []  # proj groups of the previous q-block
                    for j in range(TN):
                        nk = 4 * j + 4
                        np_ = nk // 2
                        pairs = [(h, p) for h in range(HPG) for p in range(np_)]
                        stride = max(1, len(pairs) // 16) if d_pending else 0

                        # diagonal blocks (i >= 4j): columns below 128*(i-4j)
                        # are entirely causal-masked -> skip them everywhere
                        def off_of(i):
                            return P * (i - 4 * j) if i >= 4 * j else 0

                        otj = otjp.tile([P, HPG, FN], F16, tag="otj", name=f"otj{j}")
                        otj_tiles[j] = otj
                        state = {}  # h -> (ptt, acc, po)

                        def emit_S_pair(kp):
                            h, p = pairs[kp]
                            pp = cps.tile(
                                [P, 2, FN], F32, tag="pp", name=f"pp{j}_{kp}"
                            )
                            for t in range(2):
                                i = 2 * p + t
                                off = off_of(i)
                                nc.tensor.matmul(
                                    pp[:, t, off:],
                                    qkT[:, HPG + h, i * P:(i + 1) * P],
                                    qkT[:, h, j * FN + off:(j + 1) * FN],
                                    start=True, stop=True,
                                )
                            return pp

                        psq = [emit_S_pair(kp) for kp in range(min(LOOKP, len(pairs)))]
                        for kp, (h, p) in enumerate(pairs):
                            if kp + LOOKP < len(pairs):
                                psq.append(emit_S_pair(kp + LOOKP))
                            pp = psq.pop(0)
                            i0 = 2 * p
                            if p == 0:
                                ptt = pttp.tile(
                                    [P, nk, FN], F16, tag="ptt", name=f"ptt{j}_{h}"
                                )
                                acc = accp.tile(
                                    [P, FN], F16, tag="acc", name=f"acc{j}_{h}"
                                )
                                po = ops_pool.tile(
                                    [P, FN], F32, tag="po", name=f"po{j}_{h}"
                                )
                                state[h] = (ptt, acc, po)
                            else:
                                ptt, acc, po = state[h]
                            # P^T = exp(S^T * scale); logits ~ N(0,1) in fp32
                            # PSUM so no max-subtraction needed. Clean (fully
                            # sub-diagonal) pairs take one wide exp.
                            if i0 + 1 < 4 * j:
                                nc.scalar.activation(
                                    ptt[:, i0:i0 + 2, :], pp[:, :, :], Exp,
                                    scale=SCALE,
                                )
                            else:
                                for t in range(2):
                                    i = i0 + t
                                    off = off_of(i)
                                    nc.scalar.activation(
                                        ptt[:, i, off:], pp[:, t, off:], Exp,
                                        scale=SCALE,
                                    )
                            for t in range(2):
                                i = i0 + t
                                off = off_of(i)
                                if i >= 4 * j:  # triangular diag strip
                                    nc.vector.tensor_mul(
                                        ptt[:, i, off:off + P],
                                        ptt[:, i, off:off + P],
                                        masks_sb[:, :],
                                    )
                                if i == 0:
                                    nc.vector.tensor_copy(acc[:], ptt[:, 0, :])
                                else:
                                    nc.vector.tensor_add(
                                        acc[:, off:], acc[:, off:], ptt[:, i, off:]
                                    )
                                nc.tensor.matmul(
                                    po[:, off:], vres[:, i, h, :], ptt[:, i, off:],
                                    start=(i == 0), stop=(i == nk - 1),
                                    skip_group_check=True,
                                )
                            if p == np_ - 1:
                                # r[q] summed over partitions AND broadcast to
                                # all of them by one ones-matmul; fast approx
                                # reciprocal (18 bits), then normalize O^T.
                                rbc_t = cps.tile(
                                    [P, 2, FN], F32, tag="pp", name=f"rbc{j}_{h}"
                                )
                                rbc = rbc_t[:, 0, :]
                                nc.tensor.matmul(
                                    rbc[:, :], ones16[:], acc[:],
                                    start=True, stop=True,
                                )
                                rsb = redp.tile(
                                    [P, FN], F32, tag="red", name=f"rsb{j}_{h}"
                                )
                                nc.scalar.copy(rsb[:], rbc[:, :])
                                rinv = redp.tile(
                                    [P, FN], F32, tag="rinv", name=f"rinv{j}_{h}"
                                )
                                nc.vector.reciprocal_approx_fast(rinv[:], rsb[:])
                                posb = redp.tile(
                                    [P, FN], F32, tag="posb", name=f"posb{j}_{h}"
                                )
                                nc.vector.tensor_copy(posb[:], po[:])
                                nc.vector.tensor_mul(otj[:, h, :], posb[:], rinv[:])
                            if stride and kp % stride == stride - 1 and d_pending:
                                emit_dgroup(*d_pending.pop(0))
                        while d_pending:
                            emit_dgroup(*d_pending.pop(0))
                        d_pending = [(j, g) for g in range(16)]
                    while d_pending:
                        emit_dgroup(*d_pending.pop(0))
    nc.compile()
    return nc


def _build_masks() -> np.ndarray:
    """Multiplicative causal triangle: masks[r, c] = 1.0 iff c >= r.

    A diagonal S^T tile (k-chunk i, q-chunk j) is fully masked for
    columns c < off = 128*(i-4j) (skipped outright) and triangular on
    [off, off+128): valid iff (off + c') - r >= off, i.e. c' >= r.
    """
    rr = np.arange(P)[:, None]
    cc = np.arange(P)[None, :]
    return (cc >= rr).astype(np.float16)


_CACHE: dict = {}


def _get_nc() -> bass.Bass:
    if "nc" not in _CACHE:
        _CACHE["nc"] = build_nc()
    return _CACHE["nc"]


def _make_in_maps(x, w_qkv, w_proj):
    x = np.asarray(x, dtype=np.float32)
    w_qkv = np.asarray(w_qkv, dtype=np.float32)
    w_proj = np.asarray(w_proj, dtype=np.float32)
    masks = _build_masks()
    in_maps = []
    for core in range(N_CORES):
        b, g = divmod(core, G)
        wq = w_qkv[:, DG * g:DG * (g + 1)]
        wk = w_qkv[:, C + DG * g:C + DG * (g + 1)]
        wvs = w_qkv[:, 2 * C + DG * g:2 * C + DG * (g + 1)]
        w_qk = np.concatenate([wq, wk], axis=1)  # [C, 2048]
        # [ci*128+p, m*128+col] -> [m, p, ci*128+col]
        wqk_perm = np.ascontiguousarray(
            w_qk.reshape(CI, P, MQK, P).transpose(2, 1, 0, 3).reshape(MQK, P, CI * P)
        ).astype(np.float16)
        # [ci*128+p, nv*512+vc] -> [nv, p, ci*512+vc]
        wv_perm = np.ascontiguousarray(
            wvs.reshape(CI, P, NV, FN).transpose(2, 1, 0, 3).reshape(NV, P, CI * FN)
        ).astype(np.float16)
        wpg = w_proj[DG * g:DG * (g + 1), :]  # [1024, 2048]
        wp_perm = np.ascontiguousarray(
            wpg.reshape(HPG, P, C).transpose(1, 0, 2)  # [p, hh, n]
        ).astype(np.float16)
        in_maps.append({
            "xt": np.ascontiguousarray(x[b].T).astype(np.float16),
            "wqk": wqk_perm,
            "wv": wv_perm,
            "wp": wp_perm,
            "masks": masks,
        })
    return in_maps


def run_spmd(x, w_qkv, w_proj, trace: bool = False):
    """Returns (out [B,T,C] fp32, BassKernelResults)."""
    in_maps = _make_in_maps(x, w_qkv, w_proj)
    kr = run_bass_kernel_spmd(_get_nc(), in_maps, list(range(N_CORES)), trace=trace)
    res = kr.results
    out = np.empty((B, T, C), dtype=np.float32)
    for b in range(B):
        out[b] = res[G * b]["out"] + res[G * b + 1]["out"]
    return out, kr


def kernel(x, w_qkv, w_proj) -> np.ndarray:
    out, _ = run_spmd(x, w_qkv, w_proj, trace=False)
    return out


# revision 25
# speedup vs baseline: 1.0112x; 1.0112x over previous
"""Causal self-attention on 8 TRN2 NeuronCores.

Sharding: 8 cores = 4 batches x 2 head-groups (data parallel on B,
tensor parallel on heads). Core (b, g) computes batch b, heads
8g..8g+7 end-to-end (qkv slice -> causal attention -> partial
projection); the host sums the two per-batch partials.

v2 design (vs the f32r baseline):
- fp16 everywhere on-device (PSUM accumulation stays fp32). Halves all
  DMA and SBUF traffic at identical PE rate (1 cycle/row).
- Zero DRAM roundtrips: Q^T/K^T (qkT), V (vres), O^T (otj) and w_proj
  (wpt) all live in SBUF across phases; attention + projection read
  them directly.
- Softmax denominator without PE: DVE accumulates exp chunks into a
  [128,512] fp16 acc, one Pool partition_all_reduce broadcasts the
  cross-partition sum, DVE reciprocal + column-scale normalizes.
  Removes the 320 per-chunk "ones" matmuls (10.6% of PE work), the
  Ln/Exp ACT-table thrash (65 x 1.28us), and the rinv DMA broadcast.
- Causal mask is multiplicative-after-exp in fp16 (DVE 4x mode) rather
  than additive-in-PSUM (DVE 1x).
- Projection (phase D) is interleaved per 512-wide q-block into the
  next block's attention stream, so the PE never sits idle at a phase
  boundary; only the final q-block's projection is a (fully pipelined)
  tail.

Self-contained: hardcodes B=4, T=2048, C=2048, H=16, HD=128.
"""

import numpy as np

import concourse.bass as bass
import concourse.mybir as mybir
import concourse.tile as tile
from concourse import bacc
from concourse import bass_isa
from concourse.bass_utils import run_bass_kernel_spmd

B, T, C, H = 4, 2048, 2048, 16
HD = 128          # head dim
G = 2             # head groups (tensor parallel)
HPG = H // G      # 8 heads per core
DG = HPG * HD     # 1024 = per-core concat head dim
N_CORES = 8
SCALE = float(HD) ** -0.5

F32 = mybir.dt.float32
F16 = mybir.dt.float16

P = 128           # partitions
FN = 512          # moving free-dim per matmul (one PSUM bank of fp32)
CI = C // P       # 16 contraction chunks over C
TM = T // P       # 16 t chunks of 128
TN = T // FN      # 4 t chunks of 512
MQK = 2 * DG // P # 16 qk^T row chunks
NV = DG // FN     # 2 v column chunks of 512
NC_ = C // FN     # 4 proj col chunks of 512


def build_nc() -> bass.Bass:
    nc = bacc.Bacc()
    # Host pre-layouts (all fp16):
    #   xt   = x[b].T                        [C, T]
    #   wqk  [m, p, ci*128+col] = w_qk[ci*128+p, m*128+col]
    #   wv   [nv, p, ci*512+vc] = w_v[ci*128+p, nv*512+vc]
    #   wp   [p, hh, n]         = w_proj[g*DG + hh*128 + p, n]
    #   masks[r, mm, c] = 1.0 iff (c - r) >= 128*mm else 0.0
    xt = nc.declare_dram_parameter("xt", [C, T], F16, isOutput=False)
    wqk = nc.declare_dram_parameter("wqk", [MQK, P, CI * P], F16, isOutput=False)
    wv = nc.declare_dram_parameter("wv", [NV, P, CI * FN], F16, isOutput=False)
    wp = nc.declare_dram_parameter("wp", [P, HPG, C], F16, isOutput=False)
    masks = nc.declare_dram_parameter("masks", [P, P], F16, isOutput=False)
    out = nc.declare_dram_parameter("out", [T, C], F32, isOutput=True)

    Exp = mybir.ActivationFunctionType.Exp

    with tile.TileContext(nc) as tc:
        with (
            # resident across all phases: 64 + 32 + 16 + small = ~113 KB/partition
            tc.tile_pool(name="big", bufs=1) as big,
        ):
            qkT = big.tile([P, MQK, T], F16)        # Q^T rows m<8, K^T rows m>=8
            vres = big.tile([P, TM, HPG, HD], F16)  # V natural, [k, tm, h, d]
            masks_sb = big.tile([P, P], F16)        # triangle: 1 iff c >= r
            wpt_lo = big.tile([P, HPG, C // 2], F16)  # proj cols 0..1023
            ones16 = big.tile([P, P], F16)          # partition-sum stationary
            warm = big.tile([P, 1], F32)
            nc.gpsimd.memset(ones16[:], 1.0)
            nc.gpsimd.dma_start(masks_sb[:], masks[:, :])
            # pre-warm the Exp ACT table during the load phase so phase C
            # doesn't pay the 1.3us table load at the B->C seam
            nc.scalar.activation(warm[:], ones16[:, 0:1], Exp)
            # loaded during phase B on the gpsimd SWDGE queue
            for hh in range(HPG):
                nc.gpsimd.dma_start(wpt_lo[:, hh, :], wp[:, hh, 0:C // 2])

            # ---------- Phase B: V = x @ wv, then qk^T = wqk.T @ x.T ----------
            with (
                tc.tile_pool(name="xTp", bufs=1) as xTp,
                tc.tile_pool(name="wvp", bufs=4) as wvp,
                tc.tile_pool(name="wqp", bufs=2) as wqp,
                tc.tile_pool(name="bps", bufs=4, space="PSUM") as bps,
            ):
                xT = xTp.tile([P, CI, T], F16)  # x^T resident: 64KB/partition
                for ci in range(CI):
                    nc.sync.dma_start(xT[:, ci, :], xt[ci * P:(ci + 1) * P, :])

                # V pass: out[t, v] accumulated over ci
                for nv in range(NV):
                    wvt = {}
                    for q in range(4):  # quarter-ci tiles, ACT HWDGE queue
                        t_ = wvp.tile([P, 4, FN], F16, tag="wvt", name=f"wvt{nv}_{q}")
                        nc.scalar.dma_start(
                            t_[:],
                            wv[nv, :, q * 4 * FN:(q + 1) * 4 * FN]
                            .rearrange("p (ci n) -> p ci n", ci=4),
                        )
                        wvt[q] = t_
                    for tm in range(TM):
                        ps = bps.tile([P, FN], F32, tag="v", name=f"vps{nv}_{tm}")
                        for ci in range(CI):
                            nc.tensor.matmul(
                                ps[:], xT[:, ci, tm * P:(tm + 1) * P],
                                wvt[ci // 4][:, ci % 4, :],
                                start=(ci == 0), stop=(ci == CI - 1),
                            )
                        nc.scalar.copy(vres[:, tm, 4 * nv:4 * (nv + 1), :], ps[:])

                # qk pass: out[qkrow, t] accumulated over ci
                for m in range(MQK):
                    wq = wqp.tile([P, CI, P], F16, tag="wq", name=f"wq{m}")
                    nc.sync.dma_start(
                        wq[:], wqk[m, :, :].rearrange("p (ci n) -> p ci n", ci=CI)
                    )
                    for nt in range(TN):
                        ps = bps.tile([P, FN], F32, tag="qk")
                        for ci in range(CI):
                            nc.tensor.matmul(
                                ps[:], wq[:, ci, :], xT[:, ci, nt * FN:(nt + 1) * FN],
                                start=(ci == 0), stop=(ci == CI - 1),
                            )
                        nc.scalar.copy(qkT[:, m, nt * FN:(nt + 1) * FN], ps[:])

            # ---------- Phase C (attention) + interleaved Phase D (proj) ----------
            with (
                tc.tile_pool(name="whp", bufs=1) as whp,
                tc.tile_pool(name="pttp", bufs=2) as pttp,
                tc.tile_pool(name="accp", bufs=2) as accp,
                tc.tile_pool(name="redp", bufs=2) as redp,
                tc.tile_pool(name="otjp", bufs=2) as otjp,
                tc.tile_pool(name="dstg", bufs=4) as dstg,
                tc.tile_pool(name="cps", bufs=4, space="PSUM") as cps,
                tc.tile_pool(name="ops", bufs=2, space="PSUM") as ops_pool,
                tc.tile_pool(name="dps", bufs=2, space="PSUM") as dps,
            ):
                wpt_hi = whp.tile([P, HPG, C // 2], F16)  # proj cols 1024..2047
                for hh in range(HPG):
                    nc.gpsimd.dma_start(wpt_hi[:, hh, :], wp[:, hh, C // 2:])

                otj_tiles = {}

                def emit_dgroup(jp, g):
                    # one [128 t, 512 c] proj output tile, accumulated over heads
                    n, tmq = divmod(g, 4)
                    ps = dps.tile([P, FN], F32, tag="dg", name=f"dg{jp}_{g}")
                    wsrc = wpt_lo if n < 2 else wpt_hi
                    nn = n % 2
                    otj = otj_tiles[jp]
                    for h in range(HPG):
                        nc.tensor.matmul(
                            ps[:],
                            otj[:, h, tmq * P:(tmq + 1) * P],
                            wsrc[:, h, nn * FN:(nn + 1) * FN],
                            start=(h == 0), stop=(h == HPG - 1),
                        )
                    st = dstg.tile([P, FN], F32, tag="st", name=f"st{jp}_{g}")
                    nc.scalar.copy(st[:], ps[:])
                    tg = 4 * jp + tmq
                    nc.sync.dma_start(
                        out[tg * P:(tg + 1) * P, n * FN:(n + 1) * FN], st[:]
                    )

                with nc.allow_low_precision(
                    "fp16 softmax: exp in [0, 245], row sums < 4e3, "
                    "fp16 mantissa gives ~1e-4 relative accuracy"
                ):
                    LOOK = 3
                    d_pending = []  # proj groups of the previous q-block
                    for j in range(TN):
                        nk = 4 * j + 4
                        steps = [(h, i) for h in range(HPG) for i in range(nk)]
                        stride = max(1, len(steps) // 16) if d_pending else 0

                        # diagonal blocks (i >= 4j): columns below 128*(i-4j)
                        # are entirely causal-masked -> skip them everywhere
                        def off_of(i):
                            return P * (i - 4 * j) if i >= 4 * j else 0

                        otj = otjp.tile([P, HPG, FN], F16, tag="otj", name=f"otj{j}")
                        otj_tiles[j] = otj
                        state = {}  # h -> (ptt, acc, po)

                        def emit_S(k):
                            h, i = steps[k]
                            off = off_of(i)
                            psS = cps.tile([P, FN], F32, tag="psS", name=f"psS{j}_{k}")
                            nc.tensor.matmul(
                                psS[:, off:],
                                qkT[:, HPG + h, i * P:(i + 1) * P],
                                qkT[:, h, j * FN + off:(j + 1) * FN],
                                start=True, stop=True,
                            )
                            return psS

                        psq = [emit_S(k) for k in range(min(LOOK, len(steps)))]
                        for k, (h, i) in enumerate(steps):
                            if k + LOOK < len(steps):
                                psq.append(emit_S(k + LOOK))
                            psS = psq.pop(0)
                            off = off_of(i)
                            if i == 0:
                                ptt = pttp.tile(
                                    [P, nk, FN], F16, tag="ptt", name=f"ptt{j}_{h}"
                                )
                                acc = accp.tile(
                                    [P, FN], F16, tag="acc", name=f"acc{j}_{h}"
                                )
                                po = ops_pool.tile(
                                    [P, FN], F32, tag="po", name=f"po{j}_{h}"
                                )
                                state[h] = (ptt, acc, po)
                            else:
                                ptt, acc, po = state[h]
                            # P^T chunk = exp(S^T * scale); logits ~ N(0,1) in
                            # fp32 PSUM so no max-subtraction needed.
                            nc.scalar.activation(
                                ptt[:, i, off:], psS[:, off:], Exp, scale=SCALE
                            )
                            if i >= 4 * j:  # triangular strip straddling diag
                                nc.vector.tensor_mul(
                                    ptt[:, i, off:off + P],
                                    ptt[:, i, off:off + P],
                                    masks_sb[:, :],
                                )
                            if i == 0:
                                nc.vector.tensor_copy(acc[:], ptt[:, 0, :])
                            else:
                                nc.vector.tensor_add(
                                    acc[:, off:], acc[:, off:], ptt[:, i, off:]
                                )
                            nc.tensor.matmul(
                                po[:, off:], vres[:, i, h, :], ptt[:, i, off:],
                                start=(i == 0), stop=(i == nk - 1),
                                skip_group_check=True,
                            )
                            if i == nk - 1:
                                # r[q] summed over partitions AND broadcast to
                                # all of them by one ones-matmul; fast approx
                                # reciprocal (18 bits), then normalize O^T.
                                rbc = cps.tile(
                                    [P, FN], F32, tag="psS", name=f"rbc{j}_{h}"
                                )
                                nc.tensor.matmul(
                                    rbc[:], ones16[:], acc[:],
                                    start=True, stop=True,
                                )
                                rsb = redp.tile(
                                    [P, FN], F32, tag="red", name=f"rsb{j}_{h}"
                                )
                                nc.scalar.copy(rsb[:], rbc[:])
                                rinv = redp.tile(
                                    [P, FN], F32, tag="rinv", name=f"rinv{j}_{h}"
                                )
                                nc.vector.reciprocal_approx_fast(rinv[:], rsb[:])
                                posb = redp.tile(
                                    [P, FN], F32, tag="posb", name=f"posb{j}_{h}"
                                )
                                nc.vector.tensor_copy(posb[:], po[:])
                                nc.vector.tensor_mul(otj[:, h, :], posb[:], rinv[:])
                            if stride and k % stride == stride - 1 and d_pending:
                                emit_dgroup(*d_pending.pop(0))
                        while d_pending:
                            emit_dgroup(*d_pending.pop(0))
                        d_pending = [(j, g) for g in range(16)]
                    while d_pending:
                        emit_dgroup(*d_pending.pop(0))
    nc.compile()
    return nc


def _build_masks() -> np.ndarray:
    """Multiplicative causal triangle: masks[r, c] = 1.0 iff c >= r.

    A diagonal S^T tile (k-chunk i, q-chunk j) is fully masked for
    columns c < off = 128*(i-4j) (skipped outright) and triangular on
    [off, off+128): valid iff (off + c') - r >= off, i.e. c' >= r.
    """
    rr = np.arange(P)[:, None]
    cc = np.arange(P)[None, :]
    return (cc >= rr).astype(np.float16)


_CACHE: dict = {}


def _get_nc() -> bass.Bass:
    if "nc" not in _CACHE:
        _CACHE["nc"] = build_nc()
    return _CACHE["nc"]


def _make_in_maps(x, w_qkv, w_proj):
    x = np.asarray(x, dtype=np.float32)
    w_qkv = np.asarray(w_qkv, dtype=np.float32)
    w_proj = np.asarray(w_proj, dtype=np.float32)
    masks = _build_masks()
    in_maps = []
    for core in range(N_CORES):
        b, g = divmod(core, G)
        wq = w_qkv[:, DG * g:DG * (g + 1)]
        wk = w_qkv[:, C + DG * g:C + DG * (g + 1)]
        wvs = w_qkv[:, 2 * C + DG * g:2 * C + DG * (g + 1)]
        w_qk = np.concatenate([wq, wk], axis=1)  # [C, 2048]
        # [ci*128+p, m*128+col] -> [m, p, ci*128+col]
        wqk_perm = np.ascontiguousarray(
            w_qk.reshape(CI, P, MQK, P).transpose(2, 1, 0, 3).reshape(MQK, P, CI * P)
        ).astype(np.float16)
        # [ci*128+p, nv*512+vc] -> [nv, p, ci*512+vc]
        wv_perm = np.ascontiguousarray(
            wvs.reshape(CI, P, NV, FN).transpose(2, 1, 0, 3).reshape(NV, P, CI * FN)
        ).astype(np.float16)
        wpg = w_proj[DG * g:DG * (g + 1), :]  # [1024, 2048]
        wp_perm = np.ascontiguousarray(
            wpg.reshape(HPG, P, C).transpose(1, 0, 2)  # [p, hh, n]
        ).astype(np.float16)
        in_maps.append({
            "xt": np.ascontiguousarray(x[b].T).astype(np.float16),
            "wqk": wqk_perm,
            "wv": wv_perm,
            "wp": wp_perm,
            "masks": masks,
        })
    return in_maps


def run_spmd(x, w_qkv, w_proj, trace: bool = False):
    """Returns (out [B,T,C] fp32, BassKernelResults)."""
    in_maps = _make_in_maps(x, w_qkv, w_proj)
    kr = run_bass_kernel_spmd(_get_nc(), in_maps, list(range(N_CORES)), trace=trace)
    res = kr.results
    out = np.empty((B, T, C), dtype=np.float32)
    for b in range(B):
        out[b] = res[G * b]["out"] + res[G * b + 1]["out"]
    return out, kr


def kernel(x, w_qkv, w_proj) -> np.ndarray:
    out, _ = run_spmd(x, w_qkv, w_proj, trace=False)
    return out


# revision 28
# speedup vs baseline: 1.0147x; 1.0035x over previous
"""Causal self-attention on 8 TRN2 NeuronCores.

Sharding: 8 cores = 4 batches x 2 head-groups (data parallel on B,
tensor parallel on heads). Core (b, g) computes batch b, heads
8g..8g+7 end-to-end (qkv slice -> causal attention -> partial
projection); the host sums the two per-batch partials.

v2 design (vs the f32r baseline):
- fp16 everywhere on-device (PSUM accumulation stays fp32). Halves all
  DMA and SBUF traffic at identical PE rate (1 cycle/row).
- Zero DRAM roundtrips: Q^T/K^T (qkT), V (vres), O^T (otj) and w_proj
  (wpt) all live in SBUF across phases; attention + projection read
  them directly.
- Softmax denominator at near-zero cost: DVE accumulates exp chunks
  into a [128,512] fp16 acc; ONE ones[128,128] matmul then sums it
  across partitions AND broadcasts the row sums to every partition in
  a single 512-cycle PE op; reciprocal_approx_fast (18 bits, 5x
  faster than DVE reciprocal) + a column-scale normalize O^T. This
  replaces the baseline's 320 per-chunk "ones" matmuls (10.6% of PE
  work), its Ln/Exp ACT-table thrash (65 x 1.28us), and its rinv DMA
  broadcast roundtrip.
- Causal handling: diagonal blocks skip the fully-masked column range
  [0, 128*(i-4j)) in the S matmul, exp, accumulate and O matmul
  (-20% attention work); only the 128-wide triangular strip needs a
  multiplicative fp16 mask after exp.
- Projection (phase D) is interleaved per 512-wide q-block into the
  next block's attention stream, so the PE never sits idle at a phase
  boundary; only the final q-block's projection is a (fully pipelined)
  tail.
- Engine balance: ACT (exp is the attention region's pacing engine)
  keeps the PSUM->SBUF stage copies; the per-head normalize copy runs
  on DVE; Exp's ACT table is pre-warmed during the load phase.

Self-contained: hardcodes B=4, T=2048, C=2048, H=16, HD=128.
"""

import numpy as np

import concourse.bass as bass
import concourse.mybir as mybir
import concourse.tile as tile
from concourse import bacc
from concourse.bass_utils import run_bass_kernel_spmd

B, T, C, H = 4, 2048, 2048, 16
HD = 128          # head dim
G = 2             # head groups (tensor parallel)
HPG = H // G      # 8 heads per core
DG = HPG * HD     # 1024 = per-core concat head dim
N_CORES = 8
SCALE = float(HD) ** -0.5

F32 = mybir.dt.float32
F16 = mybir.dt.float16

P = 128           # partitions
FN = 512          # moving free-dim per matmul (one PSUM bank of fp32)
CI = C // P       # 16 contraction chunks over C
TM = T // P       # 16 t chunks of 128
TN = T // FN      # 4 t chunks of 512
MQK = 2 * DG // P # 16 qk^T row chunks
NV = DG // FN     # 2 v column chunks of 512
NC_ = C // FN     # 4 proj col chunks of 512


def build_nc() -> bass.Bass:
    nc = bacc.Bacc()
    # Host pre-layouts (all fp16):
    #   xt   = x[b].T                        [C, T]
    #   wqk  [m, p, ci*128+col] = w_qk[ci*128+p, m*128+col]
    #   wv   [nv, p, ci*512+vc] = w_v[ci*128+p, nv*512+vc]
    #   wp   [p, hh, n]         = w_proj[g*DG + hh*128 + p, n]
    #   masks[r, c] = 1.0 iff c >= r (diagonal triangle)
    xt = nc.declare_dram_parameter("xt", [C, T], F16, isOutput=False)
    wqk = nc.declare_dram_parameter("wqk", [MQK, P, CI * P], F16, isOutput=False)
    wv = nc.declare_dram_parameter("wv", [NV, P, CI * FN], F16, isOutput=False)
    wp = nc.declare_dram_parameter("wp", [P, HPG, C], F16, isOutput=False)
    masks = nc.declare_dram_parameter("masks", [P, P], F16, isOutput=False)
    out = nc.declare_dram_parameter("out", [T, C], F32, isOutput=True)

    Exp = mybir.ActivationFunctionType.Exp

    with tile.TileContext(nc) as tc:
        with (
            # resident across all phases: 64 + 32 + 16 + small = ~113 KB/partition
            tc.tile_pool(name="big", bufs=1) as big,
        ):
            qkT = big.tile([P, MQK, T], F16)        # Q^T rows m<8, K^T rows m>=8
            vres = big.tile([P, TM, HPG, HD], F16)  # V natural, [k, tm, h, d]
            masks_sb = big.tile([P, P], F16)        # triangle: 1 iff c >= r
            wpt_lo = big.tile([P, HPG, C // 2], F16)  # proj cols 0..1023
            ones16 = big.tile([P, P], F16)          # partition-sum stationary
            warm = big.tile([P, 1], F32)
            nc.gpsimd.memset(ones16[:], 1.0)
            nc.gpsimd.dma_start(masks_sb[:], masks[:, :])
            # pre-warm the Exp ACT table during the load phase so phase C
            # doesn't pay the 1.3us table load at the B->C seam
            nc.scalar.activation(warm[:], ones16[:, 0:1], Exp)
            # loaded during phase B on the gpsimd SWDGE queue
            for hh in range(HPG):
                nc.gpsimd.dma_start(wpt_lo[:, hh, :], wp[:, hh, 0:C // 2])

            # ---------- Phase B: V = x @ wv, then qk^T = wqk.T @ x.T ----------
            with (
                tc.tile_pool(name="xTp", bufs=1) as xTp,
                tc.tile_pool(name="wvp", bufs=4) as wvp,
                tc.tile_pool(name="wqp", bufs=2) as wqp,
                tc.tile_pool(name="bps", bufs=4, space="PSUM") as bps,
            ):
                xT = xTp.tile([P, CI, T], F16)  # x^T resident: 64KB/partition
                for ci in range(CI):
                    nc.sync.dma_start(xT[:, ci, :], xt[ci * P:(ci + 1) * P, :])

                # V pass: out[t, v] accumulated over ci
                for nv in range(NV):
                    wvt = {}
                    for q in range(4):  # quarter-ci tiles, ACT HWDGE queue
                        t_ = wvp.tile([P, 4, FN], F16, tag="wvt", name=f"wvt{nv}_{q}")
                        nc.scalar.dma_start(
                            t_[:],
                            wv[nv, :, q * 4 * FN:(q + 1) * 4 * FN]
                            .rearrange("p (ci n) -> p ci n", ci=4),
                        )
                        wvt[q] = t_
                    for tm in range(TM):
                        ps = bps.tile([P, FN], F32, tag="v", name=f"vps{nv}_{tm}")
                        for ci in range(CI):
                            nc.tensor.matmul(
                                ps[:], xT[:, ci, tm * P:(tm + 1) * P],
                                wvt[ci // 4][:, ci % 4, :],
                                start=(ci == 0), stop=(ci == CI - 1),
                            )
                        nc.scalar.copy(vres[:, tm, 4 * nv:4 * (nv + 1), :], ps[:])

                # qk pass: out[qkrow, t] accumulated over ci
                for m in range(MQK):
                    wq = wqp.tile([P, CI, P], F16, tag="wq", name=f"wq{m}")
                    nc.sync.dma_start(
                        wq[:], wqk[m, :, :].rearrange("p (ci n) -> p ci n", ci=CI)
                    )
                    for nt in range(TN):
                        ps = bps.tile([P, FN], F32, tag="qk")
                        for ci in range(CI):
                            nc.tensor.matmul(
                                ps[:], wq[:, ci, :], xT[:, ci, nt * FN:(nt + 1) * FN],
                                start=(ci == 0), stop=(ci == CI - 1),
                            )
                        nc.scalar.copy(qkT[:, m, nt * FN:(nt + 1) * FN], ps[:])

            # ---------- Phase C (attention) + interleaved Phase D (proj) ----------
            with (
                tc.tile_pool(name="whp", bufs=1) as whp,
                tc.tile_pool(name="pttp", bufs=2) as pttp,
                tc.tile_pool(name="accp", bufs=2) as accp,
                tc.tile_pool(name="redp", bufs=2) as redp,
                tc.tile_pool(name="otjp", bufs=2) as otjp,
                tc.tile_pool(name="dstg", bufs=4) as dstg,
                tc.tile_pool(name="cps", bufs=4, space="PSUM") as cps,
                tc.tile_pool(name="ops", bufs=2, space="PSUM") as ops_pool,
                tc.tile_pool(name="dps", bufs=2, space="PSUM") as dps,
            ):
                wpt_hi = whp.tile([P, HPG, C // 2], F16)  # proj cols 1024..2047
                for hh in range(HPG):
                    nc.gpsimd.dma_start(wpt_hi[:, hh, :], wp[:, hh, C // 2:])

                otj_tiles = {}

                def emit_dgroup(jp, g):
                    # one [128 t, 512 c] proj output tile, accumulated over heads
                    n, tmq = divmod(g, 4)
                    ps = dps.tile([P, FN], F32, tag="dg", name=f"dg{jp}_{g}")
                    wsrc = wpt_lo if n < 2 else wpt_hi
                    nn = n % 2
                    otj = otj_tiles[jp]
                    for h in range(HPG):
                        nc.tensor.matmul(
                            ps[:],
                            otj[:, h, tmq * P:(tmq + 1) * P],
                            wsrc[:, h, nn * FN:(nn + 1) * FN],
                            start=(h == 0), stop=(h == HPG - 1),
                        )
                    st = dstg.tile([P, FN], F32, tag="st", name=f"st{jp}_{g}")
                    nc.scalar.copy(st[:], ps[:])
                    tg = 4 * jp + tmq
                    nc.sync.dma_start(
                        out[tg * P:(tg + 1) * P, n * FN:(n + 1) * FN], st[:]
                    )

                with nc.allow_low_precision(
                    "fp16 softmax: exp in [0, 245], row sums < 4e3, "
                    "fp16 mantissa gives ~1e-4 relative accuracy"
                ):
                    LOOK = 3
                    d_pending = []  # proj groups of the previous q-block
                    for j in range(TN):
                        nk = 4 * j + 4
                        steps = [(h, i) for h in range(HPG) for i in range(nk)]
                        stride = max(1, len(steps) // 16) if d_pending else 0

                        # diagonal blocks (i >= 4j): columns below 128*(i-4j)
                        # are entirely causal-masked -> skip them everywhere
                        def off_of(i):
                            return P * (i - 4 * j) if i >= 4 * j else 0

                        otj = otjp.tile([P, HPG, FN], F16, tag="otj", name=f"otj{j}")
                        otj_tiles[j] = otj
                        state = {}  # h -> (ptt, acc, po)

                        def emit_S(k):
                            h, i = steps[k]
                            off = off_of(i)
                            psS = cps.tile([P, FN], F32, tag="psS", name=f"psS{j}_{k}")
                            nc.tensor.matmul(
                                psS[:, off:],
                                qkT[:, HPG + h, i * P:(i + 1) * P],
                                qkT[:, h, j * FN + off:(j + 1) * FN],
                                start=True, stop=True,
                            )
                            return psS

                        psq = [emit_S(k) for k in range(min(LOOK, len(steps)))]
                        for k, (h, i) in enumerate(steps):
                            if k + LOOK < len(steps):
                                psq.append(emit_S(k + LOOK))
                            psS = psq.pop(0)
                            off = off_of(i)
                            if i == 0:
                                ptt = pttp.tile(
                                    [P, nk, FN], F16, tag="ptt", name=f"ptt{j}_{h}"
                                )
                                acc = accp.tile(
                                    [P, FN], F16, tag="acc", name=f"acc{j}_{h}"
                                )
                                po = ops_pool.tile(
                                    [P, FN], F32, tag="po", name=f"po{j}_{h}"
                                )
                                state[h] = (ptt, acc, po)
                            else:
                                ptt, acc, po = state[h]
                            # P^T chunk = exp(S^T * scale); logits ~ N(0,1) in
                            # fp32 PSUM so no max-subtraction needed.
                            nc.scalar.activation(
                                ptt[:, i, off:], psS[:, off:], Exp, scale=SCALE
                            )
                            if i >= 4 * j:  # triangular strip straddling diag
                                nc.vector.tensor_mul(
                                    ptt[:, i, off:off + P],
                                    ptt[:, i, off:off + P],
                                    masks_sb[:, :],
                                )
                            if i == 0:
                                nc.vector.tensor_copy(acc[:], ptt[:, 0, :])
                            else:
                                nc.vector.tensor_add(
                                    acc[:, off:], acc[:, off:], ptt[:, i, off:]
                                )
                            nc.tensor.matmul(
                                po[:, off:], vres[:, i, h, :], ptt[:, i, off:],
                                start=(i == 0), stop=(i == nk - 1),
                                skip_group_check=True,
                            )
                            if i == nk - 1:
                                # r[q] summed over partitions AND broadcast to
                                # all of them by one ones-matmul; fast approx
                                # reciprocal (18 bits), then normalize O^T.
                                rbc = cps.tile(
                                    [P, FN], F32, tag="psS", name=f"rbc{j}_{h}"
                                )
                                nc.tensor.matmul(
                                    rbc[:], ones16[:], acc[:],
                                    start=True, stop=True,
                                )
                                rsb = redp.tile(
                                    [P, FN], F32, tag="red", name=f"rsb{j}_{h}"
                                )
                                nc.scalar.copy(rsb[:], rbc[:])
                                rinv = redp.tile(
                                    [P, FN], F32, tag="rinv", name=f"rinv{j}_{h}"
                                )
                                nc.vector.reciprocal_approx_fast(rinv[:], rsb[:])
                                posb = redp.tile(
                                    [P, FN], F32, tag="posb", name=f"posb{j}_{h}"
                                )
                                nc.vector.tensor_copy(posb[:], po[:])
                                nc.vector.tensor_mul(otj[:, h, :], posb[:], rinv[:])
                            if stride and k % stride == stride - 1 and d_pending:
                                emit_dgroup(*d_pending.pop(0))
                        while d_pending:
                            emit_dgroup(*d_pending.pop(0))
                        d_pending = [(j, g) for g in range(16)]
                    while d_pending:
                        emit_dgroup(*d_pending.pop(0))
    nc.compile()
    return nc


def _build_masks() -> np.ndarray:
    """Multiplicative causal triangle: masks[r, c] = 1.0 iff c >= r.

    A diagonal S^T tile (k-chunk i, q-chunk j) is fully masked for
    columns c < off = 128*(i-4j) (skipped outright) and triangular on
    [off, off+128): valid iff (off + c') - r >= off, i.e. c' >= r.
    """
    rr = np.arange(P)[:, None]
    cc = np.arange(P)[None, :]
    return (cc >= rr).astype(np.float16)


_CACHE: dict = {}


def _get_nc() -> bass.Bass:
    if "nc" not in _CACHE:
        _CACHE["nc"] = build_nc()
    return _CACHE["nc"]


def _make_in_maps(x, w_qkv, w_proj):
    x = np.asarray(x, dtype=np.float32)
    w_qkv = np.asarray(w_qkv, dtype=np.float32)
    w_proj = np.asarray(w_proj, dtype=np.float32)
    masks = _build_masks()
    in_maps = []
    for core in range(N_CORES):
        b, g = divmod(core, G)
        wq = w_qkv[:, DG * g:DG * (g + 1)]
        wk = w_qkv[:, C + DG * g:C + DG * (g + 1)]
        wvs = w_qkv[:, 2 * C + DG * g:2 * C + DG * (g + 1)]
        w_qk = np.concatenate([wq, wk], axis=1)  # [C, 2048]
        # [ci*128+p, m*128+col] -> [m, p, ci*128+col]
        wqk_perm = np.ascontiguousarray(
            w_qk.reshape(CI, P, MQK, P).transpose(2, 1, 0, 3).reshape(MQK, P, CI * P)
        ).astype(np.float16)
        # [ci*128+p, nv*512+vc] -> [nv, p, ci*512+vc]
        wv_perm = np.ascontiguousarray(
            wvs.reshape(CI, P, NV, FN).transpose(2, 1, 0, 3).reshape(NV, P, CI * FN)
        ).astype(np.float16)
        wpg = w_proj[DG * g:DG * (g + 1), :]  # [1024, 2048]
        wp_perm = np.ascontiguousarray(
            wpg.reshape(HPG, P, C).transpose(1, 0, 2)  # [p, hh, n]
        ).astype(np.float16)
        in_maps.append({
            "xt": np.ascontiguousarray(x[b].T).astype(np.float16),
            "wqk": wqk_perm,
            "wv": wv_perm,
            "wp": wp_perm,
            "masks": masks,
        })
    return in_maps


def run_spmd(x, w_qkv, w_proj, trace: bool = False):
    """Returns (out [B,T,C] fp32, BassKernelResults)."""
    in_maps = _make_in_maps(x, w_qkv, w_proj)
    kr = run_bass_kernel_spmd(_get_nc(), in_maps, list(range(N_CORES)), trace=trace)
    res = kr.results
    out = np.empty((B, T, C), dtype=np.float32)
    for b in range(B):
        out[b] = res[G * b]["out"] + res[G * b + 1]["out"]
    return out, kr


def kernel(x, w_qkv, w_proj) -> np.ndarray:
    out, _ = run_spmd(x, w_qkv, w_proj, trace=False)
    return out


# revision 33
# speedup vs baseline: 1.0232x; 1.0083x over previous
"""Causal self-attention on 8 TRN2 NeuronCores.

Sharding: 8 cores = 4 batches x 2 head-groups (data parallel on B,
tensor parallel on heads). Core (b, g) computes batch b, heads
8g..8g+7 end-to-end (qkv slice -> causal attention -> partial
projection); the host sums the two per-batch partials.

v2 design (vs the f32r baseline):
- fp16 everywhere on-device (PSUM accumulation stays fp32). Halves all
  DMA and SBUF traffic at identical PE rate (1 cycle/row).
- Zero DRAM roundtrips: Q^T/K^T (qkT), V (vres), O^T (otj) and w_proj
  (wpt) all live in SBUF across phases; attention + projection read
  them directly.
- Softmax denominator at near-zero cost: DVE accumulates exp chunks
  into a [128,512] fp16 acc; ONE ones[128,128] matmul then sums it
  across partitions AND broadcasts the row sums to every partition in
  a single 512-cycle PE op; reciprocal_approx_fast (18 bits, 5x
  faster than DVE reciprocal) + a column-scale normalize O^T. This
  replaces the baseline's 320 per-chunk "ones" matmuls (10.6% of PE
  work), its Ln/Exp ACT-table thrash (65 x 1.28us), and its rinv DMA
  broadcast roundtrip.
- Causal handling: diagonal blocks skip the fully-masked column range
  [0, 128*(i-4j)) in the S matmul, exp, accumulate and O matmul
  (-20% attention work); only the 128-wide triangular strip needs a
  multiplicative fp16 mask after exp.
- Projection (phase D) is interleaved per 512-wide q-block into the
  next block's attention stream, so the PE never sits idle at a phase
  boundary; only the final q-block's projection is a (fully pipelined)
  tail.
- Engine balance: ACT (exp is the attention region's pacing engine)
  keeps the PSUM->SBUF stage copies; the per-head normalize copy runs
  on DVE; Exp's ACT table is pre-warmed during the load phase.

Self-contained: hardcodes B=4, T=2048, C=2048, H=16, HD=128.
"""

import numpy as np

import concourse.bass as bass
import concourse.mybir as mybir
import concourse.tile as tile
from concourse import bacc
from concourse.bass_utils import run_bass_kernel_spmd

B, T, C, H = 4, 2048, 2048, 16
HD = 128          # head dim
G = 2             # head groups (tensor parallel)
HPG = H // G      # 8 heads per core
DG = HPG * HD     # 1024 = per-core concat head dim
N_CORES = 8
SCALE = float(HD) ** -0.5

F32 = mybir.dt.float32
F16 = mybir.dt.float16

P = 128           # partitions
FN = 512          # moving free-dim per matmul (one PSUM bank of fp32)
CI = C // P       # 16 contraction chunks over C
TM = T // P       # 16 t chunks of 128
TN = T // FN      # 4 t chunks of 512
MQK = 2 * DG // P # 16 qk^T row chunks
NV = DG // FN     # 2 v column chunks of 512
NC_ = C // FN     # 4 proj col chunks of 512


def build_nc() -> bass.Bass:
    nc = bacc.Bacc()
    # Host pre-layouts (all fp16):
    #   xt   = x[b].T                        [C, T]
    #   wqk  [m, p, ci*128+col] = w_qk[ci*128+p, m*128+col]
    #   wv   [nv, p, ci*512+vc] = w_v[ci*128+p, nv*512+vc]
    #   wp   [p, hh, n]         = w_proj[g*DG + hh*128 + p, n]
    #   masks[r, c] = 1.0 iff c >= r (diagonal triangle)
    # x^T shipped tm-major ([tm, c, 128]) so each V-pass token block needs
    # only its own 512KB slice — compute ramps after ~1MB instead of 8MB
    xt = nc.declare_dram_parameter("xt", [TM, C, P], F16, isOutput=False)
    wqk = nc.declare_dram_parameter("wqk", [MQK, P, CI * P], F16, isOutput=False)
    wv = nc.declare_dram_parameter("wv", [NV, P, CI * FN], F16, isOutput=False)
    wp = nc.declare_dram_parameter("wp", [P, HPG, C], F16, isOutput=False)
    masks = nc.declare_dram_parameter("masks", [P, P], F16, isOutput=False)
    out = nc.declare_dram_parameter("out", [T, C], F32, isOutput=True)

    Exp = mybir.ActivationFunctionType.Exp

    with tile.TileContext(nc) as tc:
        with (
            # resident across all phases: 64 + 32 + 16 + small = ~113 KB/partition
            tc.tile_pool(name="big", bufs=1) as big,
        ):
            qkT = big.tile([P, MQK, T], F16)        # Q^T rows m<8, K^T rows m>=8
            vres = big.tile([P, TM, HPG, HD], F16)  # V natural, [k, tm, h, d]
            masks_sb = big.tile([P, P], F16)        # triangle: 1 iff c >= r
            wpt_lo = big.tile([P, HPG, C // 2], F16)  # proj cols 0..1023
            ones16 = big.tile([P, P], F16)          # partition-sum stationary
            warm = big.tile([P, 1], F32)
            nc.gpsimd.memset(ones16[:], 1.0)
            nc.gpsimd.dma_start(masks_sb[:], masks[:, :])
            # pre-warm the Exp ACT table during the load phase so phase C
            # doesn't pay the 1.3us table load at the B->C seam
            nc.scalar.activation(warm[:], ones16[:, 0:1], Exp)
            # loaded during phase B on the gpsimd SWDGE queue
            for hh in range(HPG):
                nc.gpsimd.dma_start(wpt_lo[:, hh, :], wp[:, hh, 0:C // 2])

            # ---------- Phase B: V = x @ wv, then qk^T = wqk.T @ x.T ----------
            with (
                tc.tile_pool(name="xTp", bufs=1) as xTp,
                tc.tile_pool(name="wvp", bufs=4) as wvp,
                tc.tile_pool(name="wqp", bufs=2) as wqp,
                tc.tile_pool(name="bps", bufs=4, space="PSUM") as bps,
            ):
                xT = xTp.tile([P, TM, CI, P], F16)  # x^T resident: 64KB/partition
                for tm in range(TM):
                    nc.sync.dma_start(
                        xT[:, tm, :, :],
                        xt[tm, :, :].rearrange("(ci p) t -> p ci t", p=P),
                    )

                # V pass: out[t, v] accumulated over ci
                for nv in range(NV):
                    wvt = {}
                    for q in range(4):  # quarter-ci tiles, ACT HWDGE queue
                        t_ = wvp.tile([P, 4, FN], F16, tag="wvt", name=f"wvt{nv}_{q}")
                        nc.scalar.dma_start(
                            t_[:],
                            wv[nv, :, q * 4 * FN:(q + 1) * 4 * FN]
                            .rearrange("p (ci n) -> p ci n", ci=4),
                        )
                        wvt[q] = t_
                    for tm in range(TM):
                        ps = bps.tile([P, FN], F32, tag="v", name=f"vps{nv}_{tm}")
                        for ci in range(CI):
                            nc.tensor.matmul(
                                ps[:], xT[:, tm, ci, :],
                                wvt[ci // 4][:, ci % 4, :],
                                start=(ci == 0), stop=(ci == CI - 1),
                            )
                        nc.scalar.copy(vres[:, tm, 4 * nv:4 * (nv + 1), :], ps[:])

                # qk pass: out[qkrow, t] accumulated over ci
                for m in range(MQK):
                    wq = wqp.tile([P, CI, P], F16, tag="wq", name=f"wq{m}")
                    nc.sync.dma_start(
                        wq[:], wqk[m, :, :].rearrange("p (ci n) -> p ci n", ci=CI)
                    )
                    for nt in range(TN):
                        ps = bps.tile([P, FN], F32, tag="qk")
                        for ci in range(CI):
                            nc.tensor.matmul(
                                ps[:], wq[:, ci, :],
                                xT[:, nt * 4:(nt + 1) * 4, ci, :],
                                start=(ci == 0), stop=(ci == CI - 1),
                            )
                        nc.scalar.copy(qkT[:, m, nt * FN:(nt + 1) * FN], ps[:])

            # ---------- Phase C (attention) + interleaved Phase D (proj) ----------
            with (
                tc.tile_pool(name="whp", bufs=1) as whp,
                tc.tile_pool(name="pttp", bufs=2) as pttp,
                tc.tile_pool(name="accp", bufs=2) as accp,
                tc.tile_pool(name="redp", bufs=2) as redp,
                tc.tile_pool(name="otjp", bufs=2) as otjp,
                tc.tile_pool(name="dstg", bufs=4) as dstg,
                tc.tile_pool(name="cps", bufs=4, space="PSUM") as cps,
                tc.tile_pool(name="ops", bufs=2, space="PSUM") as ops_pool,
                tc.tile_pool(name="dps", bufs=2, space="PSUM") as dps,
            ):
                wpt_hi = whp.tile([P, HPG, C // 2], F16)  # proj cols 1024..2047
                for hh in range(HPG):
                    nc.gpsimd.dma_start(wpt_hi[:, hh, :], wp[:, hh, C // 2:])

                otj_tiles = {}

                def emit_dgroup(jp, g):
                    # one [128 t, 512 c] proj output tile, accumulated over heads
                    n, tmq = divmod(g, 4)
                    ps = dps.tile([P, FN], F32, tag="dg", name=f"dg{jp}_{g}")
                    wsrc = wpt_lo if n < 2 else wpt_hi
                    nn = n % 2
                    otj = otj_tiles[jp]
                    for h in range(HPG):
                        nc.tensor.matmul(
                            ps[:],
                            otj[:, h, tmq * P:(tmq + 1) * P],
                            wsrc[:, h, nn * FN:(nn + 1) * FN],
                            start=(h == 0), stop=(h == HPG - 1),
                        )
                    st = dstg.tile([P, FN], F32, tag="st", name=f"st{jp}_{g}")
                    nc.scalar.copy(st[:], ps[:])
                    tg = 4 * jp + tmq
                    nc.sync.dma_start(
                        out[tg * P:(tg + 1) * P, n * FN:(n + 1) * FN], st[:]
                    )

                with nc.allow_low_precision(
                    "fp16 softmax: exp in [0, 245], row sums < 4e3, "
                    "fp16 mantissa gives ~1e-4 relative accuracy"
                ):
                    LOOK = 3
                    d_pending = []  # proj groups of the previous q-block
                    for j in range(TN):
                        nk = 4 * j + 4
                        steps = [(h, i) for h in range(HPG) for i in range(nk)]
                        stride = max(1, len(steps) // 16) if d_pending else 0

                        # diagonal blocks (i >= 4j): columns below 128*(i-4j)
                        # are entirely causal-masked -> skip them everywhere
                        def off_of(i):
                            return P * (i - 4 * j) if i >= 4 * j else 0

                        otj = otjp.tile([P, HPG, FN], F16, tag="otj", name=f"otj{j}")
                        otj_tiles[j] = otj
                        state = {}  # h -> (ptt, acc, po)

                        def emit_S(k):
                            h, i = steps[k]
                            off = off_of(i)
                            psS = cps.tile([P, FN], F32, tag="psS", name=f"psS{j}_{k}")
                            nc.tensor.matmul(
                                psS[:, off:],
                                qkT[:, HPG + h, i * P:(i + 1) * P],
                                qkT[:, h, j * FN + off:(j + 1) * FN],
                                start=True, stop=True,
                            )
                            return psS

                        psq = [emit_S(k) for k in range(min(LOOK, len(steps)))]
                        for k, (h, i) in enumerate(steps):
                            if k + LOOK < len(steps):
                                psq.append(emit_S(k + LOOK))
                            psS = psq.pop(0)
                            off = off_of(i)
                            if i == 0:
                                ptt = pttp.tile(
                                    [P, nk, FN], F16, tag="ptt", name=f"ptt{j}_{h}"
                                )
                                acc = accp.tile(
                                    [P, FN], F16, tag="acc", name=f"acc{j}_{h}"
                                )
                                po = ops_pool.tile(
                                    [P, FN], F32, tag="po", name=f"po{j}_{h}"
                                )
                                state[h] = (ptt, acc, po)
                            else:
                                ptt, acc, po = state[h]
                            # P^T chunk = exp(S^T * scale); logits ~ N(0,1) in
                            # fp32 PSUM so no max-subtraction needed.
                            nc.scalar.activation(
                                ptt[:, i, off:], psS[:, off:], Exp, scale=SCALE
                            )
                            if i >= 4 * j:  # triangular strip straddling diag
                                nc.vector.tensor_mul(
                                    ptt[:, i, off:off + P],
                                    ptt[:, i, off:off + P],
                                    masks_sb[:, :],
                                )
                            if i == 0:
                                nc.vector.tensor_copy(acc[:], ptt[:, 0, :])
                            else:
                                nc.vector.tensor_add(
                                    acc[:, off:], acc[:, off:], ptt[:, i, off:]
                                )
                            nc.tensor.matmul(
                                po[:, off:], vres[:, i, h, :], ptt[:, i, off:],
                                start=(i == 0), stop=(i == nk - 1),
                                skip_group_check=True,
                            )
                            if i == nk - 1:
                                # r[q] summed over partitions AND broadcast to
                                # all of them by one ones-matmul; fast approx
                                # reciprocal (18 bits), then normalize O^T.
                                rbc = cps.tile(
                                    [P, FN], F32, tag="psS", name=f"rbc{j}_{h}"
                                )
                                nc.tensor.matmul(
                                    rbc[:], ones16[:], acc[:],
                                    start=True, stop=True,
                                )
                                rsb = redp.tile(
                                    [P, FN], F32, tag="red", name=f"rsb{j}_{h}"
                                )
                                nc.scalar.copy(rsb[:], rbc[:])
                                rinv = redp.tile(
                                    [P, FN], F32, tag="rinv", name=f"rinv{j}_{h}"
                                )
                                nc.vector.reciprocal_approx_fast(rinv[:], rsb[:])
                                posb = redp.tile(
                                    [P, FN], F32, tag="posb", name=f"posb{j}_{h}"
                                )
                                nc.vector.tensor_copy(posb[:], po[:])
                                nc.vector.tensor_mul(otj[:, h, :], posb[:], rinv[:])
                            if stride and k % stride == stride - 1 and d_pending:
                                emit_dgroup(*d_pending.pop(0))
                        while d_pending:
                            emit_dgroup(*d_pending.pop(0))
                        d_pending = [(j, g) for g in range(16)]
                    while d_pending:
                        emit_dgroup(*d_pending.pop(0))
    nc.compile()
    return nc


def _build_masks() -> np.ndarray:
    """Multiplicative causal triangle: masks[r, c] = 1.0 iff c >= r.

    A diagonal S^T tile (k-chunk i, q-chunk j) is fully masked for
    columns c < off = 128*(i-4j) (skipped outright) and triangular on
    [off, off+128): valid iff (off + c') - r >= off, i.e. c' >= r.
    """
    rr = np.arange(P)[:, None]
    cc = np.arange(P)[None, :]
    return (cc >= rr).astype(np.float16)


_CACHE: dict = {}


def _get_nc() -> bass.Bass:
    if "nc" not in _CACHE:
        _CACHE["nc"] = build_nc()
    return _CACHE["nc"]


def _make_in_maps(x, w_qkv, w_proj):
    x = np.asarray(x, dtype=np.float32)
    w_qkv = np.asarray(w_qkv, dtype=np.float32)
    w_proj = np.asarray(w_proj, dtype=np.float32)
    masks = _build_masks()
    in_maps = []
    for core in range(N_CORES):
        b, g = divmod(core, G)
        wq = w_qkv[:, DG * g:DG * (g + 1)]
        wk = w_qkv[:, C + DG * g:C + DG * (g + 1)]
        wvs = w_qkv[:, 2 * C + DG * g:2 * C + DG * (g + 1)]
        w_qk = np.concatenate([wq, wk], axis=1)  # [C, 2048]
        # [ci*128+p, m*128+col] -> [m, p, ci*128+col]
        wqk_perm = np.ascontiguousarray(
            w_qk.reshape(CI, P, MQK, P).transpose(2, 1, 0, 3).reshape(MQK, P, CI * P)
        ).astype(np.float16)
        # [ci*128+p, nv*512+vc] -> [nv, p, ci*512+vc]
        wv_perm = np.ascontiguousarray(
            wvs.reshape(CI, P, NV, FN).transpose(2, 1, 0, 3).reshape(NV, P, CI * FN)
        ).astype(np.float16)
        wpg = w_proj[DG * g:DG * (g + 1), :]  # [1024, 2048]
        wp_perm = np.ascontiguousarray(
            wpg.reshape(HPG, P, C).transpose(1, 0, 2)  # [p, hh, n]
        ).astype(np.float16)
        xt_tm = np.ascontiguousarray(
            x[b].T.reshape(C, TM, P).transpose(1, 0, 2)  # [tm, c, 128]
        ).astype(np.float16)
        in_maps.append({
            "xt": xt_tm,
            "wqk": wqk_perm,
            "wv": wv_perm,
            "wp": wp_perm,
            "masks": masks,
        })
    return in_maps


def run_spmd(x, w_qkv, w_proj, trace: bool = False):
    """Returns (out [B,T,C] fp32, BassKernelResults)."""
    in_maps = _make_in_maps(x, w_qkv, w_proj)
    kr = run_bass_kernel_spmd(_get_nc(), in_maps, list(range(N_CORES)), trace=trace)
    res = kr.results
    out = np.empty((B, T, C), dtype=np.float32)
    for b in range(B):
        out[b] = res[G * b]["out"] + res[G * b + 1]["out"]
    return out, kr


def kernel(x, w_qkv, w_proj) -> np.ndarray:
    out, _ = run_spmd(x, w_qkv, w_proj, trace=False)
    return out
